# revision 15
# baseline (speedup 1.0000x reference)
"""CPCNet forward on 8 Trainium2 NeuronCores (Bass/Tile).

Data-parallel over batch: each of the 8 cores processes 16 of the 128
batch elements end-to-end (embed GEMM -> GRU over 16 context windows ->
bilinear scoring), parameters replicated. No collectives needed.

The embed GEMM dominates (memory regime). Two host-side preprocessing
moves cut the device work to a pure DMA stream:
  1. X is cast f32->bf16 on the host, halving HBM traffic
     (103 MB -> 52 MB per core; rel err of bf16 embed ~3e-3).
  2. X is pre-transposed on the host into partition-major chunk layout
     X^T[p, j*rows + r] = X[r, j*128 + p], so the contraction dim (CT)
     lands on partitions. The kernel then needs NO on-chip transposes
     and no PSUM-evacuation copies: the PE just streams 66 chunk
     matmuls per row-group, accumulating E^T directly in PSUM banks.

The embed bias is folded into the GEMM as a virtual 8401st input
column: chunk 65 partition 80 of X^T is 1.0 and the matching W row is
b_embed, so E^T lands in PSUM bias-included.

Per-core layout: rows = [Xc 256 (s*16+b) | Xp 256 | Xb 2560 (nb,s,b)].
Stream 1 (Xc+Xp, 8.45 MB, one DMA) accumulates into PSUM bank A so the
GRU can start ~30 us in; stream 2 (Xb, 43.3 MB, 33 two-chunk DMAs)
accumulates into 5 more PSUM banks while the GRU + bilinear A-matrices
run in the PE's idle gaps. Scoring is a DVE broadcast-multiply against
A plus a ones-matmul column reduction (float32r) at the tail.
"""

import numpy as np

import concourse.bacc as bacc
import concourse.mybir as mybir
import concourse.tile as tile
from concourse.tile import add_dep_helper
from concourse.bass_utils import run_bass_kernel_spmd

N_CORES = 8
BC = 16          # batch per core
NE = 16          # context windows (gru seq len)
NB = 10          # negative samples
CT = 8400        # flattened window (21*400)
E = 100          # embed dim == gru hidden
NCHUNK = 66      # ceil(8448/128); chunk 65 rows 0..79 real, row 80 bias
R1 = 2 * NE * BC            # 512 rows: Xc + Xp
R2 = NB * NE * BC           # 2560 rows: Xb
NPAIR = NCHUNK // 2         # stream-2 DMAs carry 2 chunks each

F32 = mybir.dt.float32
F32R = mybir.dt.float32r
BF16 = mybir.dt.bfloat16


def _emit(nc, tc, ctx):
    X1 = nc.dram_tensor("X1", [128, NCHUNK * R1], BF16, kind="ExternalInput").ap()
    X2 = nc.dram_tensor("X2", [128, NCHUNK * R2], BF16, kind="ExternalInput").ap()
    Wemb = nc.dram_tensor("Wemb", [128, NCHUNK * E], BF16,
                          kind="ExternalInput").ap()
    WihT = nc.dram_tensor("WihT", [E, 300], F32, kind="ExternalInput").ap()
    WhhT = nc.dram_tensor("WhhT", [E, 300], F32, kind="ExternalInput").ap()
    bias4 = nc.dram_tensor("bias4", [E, 4], F32, kind="ExternalInput").ap()
    Wbil = nc.dram_tensor("Wbil", [E, NE * E], F32, kind="ExternalInput").ap()
    ones = nc.dram_tensor("ones", [E, 1], F32R, kind="ExternalInput").ap()
    out_d = nc.dram_tensor("out", [1, R1 // 2 + R2], F32,
                           kind="ExternalOutput").ap()

    P = ctx.enter_context  # pools

    const = P(tc.tile_pool(name="const", bufs=1))
    x1p = P(tc.tile_pool(name="x1p", bufs=3))
    x2p = P(tc.tile_pool(name="x2p", bufs=9))
    psA = P(tc.tile_pool(name="psA", bufs=1, space="PSUM"))
    psT = P(tc.tile_pool(name="psT", bufs=2, space="PSUM"))
    small = P(tc.tile_pool(name="small", bufs=2))

    # ---- persistent SBUF ----
    # W_embed arrives pre-chunked [128, 66*100], pre-cast to bf16, bias
    # folded into chunk 65 row 80: one contiguous 1.7 MB DMA.
    W_sb = const.tile([128, NCHUNK * E], BF16)
    nc.sync.dma_start(W_sb[:], Wemb[:])
    # small params on the other HWDGE ring (scalar) to keep sync FIFO clean
    WihT_sb = const.tile([E, 300], F32)
    nc.scalar.dma_start(WihT_sb[:], WihT[:])
    WhhT_sb = const.tile([E, 300], F32)
    nc.scalar.dma_start(WhhT_sb[:], WhhT[:])
    bias4_sb = const.tile([E, 4], F32)
    nc.scalar.dma_start(bias4_sb[:], bias4[:])
    Wbil_sb = const.tile([E, NE * E], F32)
    nc.scalar.dma_start(Wbil_sb[:], Wbil[:])
    ones_sb = const.tile([E, 1], F32R)
    nc.scalar.dma_start(ones_sb[:], ones[:])

    EcT = const.tile([E, NE * BC], F32)            # Ec^T (bias included)
    gi_sb = const.tile([E, NE * 3 * BC], F32)      # preacts, [s][r|z|n] blocks
    h = const.tile([E, BC], F32)                   # GRU hidden state (h^T)
    A_sb = const.tile([E, NE * BC], F32)           # bilinear A^T, [s][b] cols
    out_sb = const.tile([1, R1 // 2 + R2], F32)

    # Prime the ACT sigmoid/tanh tables during the X1 phase -- each
    # ACT_TABLE_LOAD costs ~1.3 us and would otherwise land mid-GRU on
    # the serial h-chain.
    scr = const.tile([1, 4], F32)
    nc.vector.memset(scr[:], 0.0)
    scr2 = const.tile([1, 4], F32)
    nc.scalar.activation(scr2[:], scr[:], mybir.ActivationFunctionType.Sigmoid)
    scr3 = const.tile([1, 4], F32)
    nc.scalar.activation(scr3[:], scr[:], mybir.ActivationFunctionType.Tanh)

    # ---- PSUM accumulators: 6 full banks, live the whole kernel ----
    bankA = psA.tile([E, R1], F32)                 # Ec^T | Ep^T
    banksB = [psA.tile([E, 512], F32, name=f"bankB{k}") for k in range(5)]

    gi_v = gi_sb.rearrange("e (s g b) -> e s g b", s=NE, g=3)

    def gru_init():
        # gi preacts for all 16 steps in 3 gate matmuls; biases folded
        # (r,z get b_ih+b_hh; n gets b_ih only).  Scattered into the
        # per-step-interleaved gi layout so each step reads one slice.
        nc.vector.memset(h[:], 0.0)
        nc.vector.tensor_copy(EcT[:], bankA[:, 0:NE * BC])
        for g in range(3):
            gp = psT.tile([128, 512], F32, tag="t", name="gp")
            nc.tensor.matmul(gp[0:E, 0:NE * BC], WihT_sb[:, g * E:(g + 1) * E],
                             EcT[:], start=True, stop=True,
                             skip_group_check=True)
            nc.scalar.add(gi_v[:, :, g, :],
                          gp[0:E, 0:NE * BC].rearrange("e (s b) -> e s b", s=NE),
                          bias4_sb[:, g:g + 1])

    def gru_step(s, after_mm=None):
        # DVE only evacuates gh (1 op); elementwise on the idle GpSimd,
        # sigmoid/tanh on ACT.
        c0 = s * 3 * BC
        gh = psT.tile([128, 512], F32, tag="t", name="gh")
        for g in range(3):
            mm = nc.tensor.matmul(gh[0:E, g * BC:(g + 1) * BC],
                                  WhhT_sb[:, g * E:(g + 1) * E], h[:],
                                  start=True, stop=True,
                                  skip_group_check=True)
            if after_mm is not None:
                # pin the step into the PE queue after its pair's matmuls
                # so the scheduler cannot clump consecutive steps (each
                # clump = ~3 us PE idle on the serial h-chain -> HAM K=4/8)
                add_dep_helper(mm.ins, after_mm.ins, sync=False,
                               reason="pin gru step after its pair")
        ghs = small.tile([E, 3 * BC], F32, tag="ghs", name="ghs")
        nc.vector.tensor_copy(ghs[:], gh[0:E, 0:3 * BC])
        rzt = small.tile([E, 2 * BC], F32, tag="rzt", name="rzt")
        nc.gpsimd.tensor_add(rzt[:], ghs[:, 0:2 * BC], gi_sb[:, c0:c0 + 2 * BC])
        rz = small.tile([E, 2 * BC], F32, tag="rz", name="rz")
        nc.scalar.activation(rz[:], rzt[:],
                             mybir.ActivationFunctionType.Sigmoid)
        hn = small.tile([E, BC], F32, tag="hn", name="hn")
        nc.gpsimd.tensor_scalar_add(hn[:], ghs[:, 2 * BC:3 * BC],
                                    bias4_sb[:, 3:4])  # gh_n + b_hn
        t1 = small.tile([E, BC], F32, tag="t1", name="t1")
        nc.gpsimd.tensor_mul(t1[:], rz[:, 0:BC], hn[:])
        t2 = small.tile([E, BC], F32, tag="t2", name="t2")
        nc.gpsimd.tensor_add(t2[:], t1[:], gi_sb[:, c0 + 2 * BC:c0 + 3 * BC])
        n = small.tile([E, BC], F32, tag="n", name="n")
        nc.scalar.activation(n[:], t2[:], mybir.ActivationFunctionType.Tanh)
        d = small.tile([E, BC], F32, tag="d", name="d")
        nc.gpsimd.tensor_sub(d[:], h[:], n[:])
        zd = small.tile([E, BC], F32, tag="zd", name="zd")
        nc.gpsimd.tensor_mul(zd[:], rz[:, BC:2 * BC], d[:])
        nc.gpsimd.tensor_add(h[:], n[:], zd[:])    # h = n + z*(h-n)

    tp = small.tile([E, NE * BC], F32R, tag="tmp0", bufs=1, name="tp")

    def bilinear_A(after_mm=None):
        # A^T[:, s*16+b] = W_bil[s].T @ h^T  for all 16 s into one bank
        Ap = psT.tile([128, 512], F32, tag="t", name="Ap")
        for s in range(NE):
            mm = nc.tensor.matmul(Ap[0:E, s * BC:(s + 1) * BC],
                                  Wbil_sb[:, s * E:(s + 1) * E], h[:],
                                  start=True, stop=True,
                                  skip_group_check=True)
            if after_mm is not None:
                add_dep_helper(mm.ins, after_mm.ins, sync=False,
                               reason="pin bilinear A after its pair")
        nc.vector.tensor_copy(A_sb[:], Ap[0:E, 0:NE * BC])
        # Ep scores can multiply immediately (bankA finished long ago);
        # overlaps the stream-2 tail on the otherwise idle DVE.
        nc.vector.tensor_mul(tp[:], bankA[:, NE * BC:2 * NE * BC], A_sb[:])

    # ---- stream 1: Xc+Xp accumulate into bankA, 6 DMAs of 11 chunks ----
    x1_dmas = []
    for p in range(6):
        xs1 = x1p.tile([128, 11 * R1], BF16, tag="x1", name="xs1")
        x1_dmas.append(nc.sync.dma_start(
            xs1[:], X1[:, p * 11 * R1:(p + 1) * 11 * R1]))
        if p >= 2:
            # cap in-flight x1 DMAs at 2: the 16 SDMA engines round-robin
            # packets of ALL queued DMAs, so deep queues smear every
            # completion signal to the end of the in-flight window
            add_dep_helper(x1_dmas[p].ins, x1_dmas[p - 2].ins, sync=True,
                           reason="cap x1 DMA in-flight depth")
        for jj in range(11):
            j = p * 11 + jj
            nc.tensor.matmul(bankA[:, :], W_sb[:, j * E:(j + 1) * E],
                             xs1[:, jj * R1:(jj + 1) * R1],
                             start=(j == 0), stop=(j == NCHUNK - 1),
                             skip_group_check=True)
    gru_init()

    # ---- stream 2: Xb, 33 two-chunk DMAs; one GRU step per pair for
    # the first 16 pairs.  DMA in-flight depth is capped at 3 so
    # completions stay near-FIFO (~3.1 us apart at line rate): each
    # pair's matmuls fire right as its own data lands, which both paces
    # the PE (hiding the ~2.9 us serial GRU h-chain) and keeps the tail
    # backlog at zero.  Deeper queues would smear every completion to
    # the end of the in-flight window (SDMA engines round-robin packets
    # of all queued DMAs) and make the PE run ~9 pairs behind. ----
    INFLIGHT = 3
    dma_insts = []
    for pi in range(NPAIR):
        xs = x2p.tile([128, 2 * R2], BF16, tag="x2", name="xs")
        dma_insts.append(nc.sync.dma_start(
            xs[:], X2[:, pi * 2 * R2:(pi + 1) * 2 * R2]))
        if pi >= INFLIGHT:
            add_dep_helper(dma_insts[pi].ins, dma_insts[pi - INFLIGHT].ins,
                           sync=True, reason="cap x2 DMA in-flight depth")
        mms = []
        for u in range(2):
            j = pi * 2 + u
            for k in range(5):
                mms.append(nc.tensor.matmul(
                    banksB[k][:, :], W_sb[:, j * E:(j + 1) * E],
                    xs[:, u * R2 + k * 512:u * R2 + (k + 1) * 512],
                    start=(j == 0), stop=(j == NCHUNK - 1),
                    skip_group_check=True))
        if 1 <= pi <= NE:
            gru_step(pi - 1, after_mm=mms[-1])
        elif pi == NE + 2:
            bilinear_A(after_mm=mms[-1])

    # ---- bilinear scores ----
    # score[r] = A[:, sb(r)] . E^T[:, r]; rows repeat the (s,b) pattern
    # every 256 cols, so the multiplier is A_sb broadcast per 256-block.
    A2 = A_sb.unsqueeze(1).broadcast_to([E, 2, NE * BC])
    SB = NE * BC
    tmps = [(tp, SB)]
    for k in range(5):
        tk = small.tile([E, 512], F32R, tag=f"tmp{k + 1}", bufs=1, name="tk")
        nc.vector.tensor_mul(tk.rearrange("e (i b) -> e i b", i=2),
                             banksB[k][:, :].rearrange("e (i b) -> e i b", i=2),
                             A2)
        tmps.append((tk, 512))
    c0 = 0
    for ti, (tk, w) in enumerate(tmps):
        rp = psT.tile([128, 512], F32, tag="t", name="rp")
        nc.tensor.matmul(rp[0:1, 0:w], ones_sb[:, 0:1], tk[:, 0:w],
                         start=True, stop=True, skip_group_check=True)
        nc.scalar.copy(out_sb[:, c0:c0 + w], rp[0:1, 0:w])
        c0 += w
    nc.sync.dma_start(out_d[:], out_sb[:])


def build():
    import contextlib
    nc = bacc.Bacc("TRN2", target_bir_lowering=False, debug=False,
                   enable_asserts=False, num_devices=N_CORES)
    with tile.TileContext(nc) as tc:
        with contextlib.ExitStack() as ctx:
            _emit(nc, tc, ctx)
    nc.compile()
    return nc


_NC = None
_PREP = None


def _get_prep():
    """jit-compiled per-core X preprocessor on the CPU backend."""
    global _PREP
    if _PREP is not None:
        return _PREP
    import jax
    import jax.numpy as jnp

    def prep(xc, xp, xb):
        # xc/xp [16b,16s,8400], xb [16b,16s,10,8400] (f32)
        rc = jnp.transpose(xc, (1, 0, 2)).reshape(NE * BC, CT)    # (s,b)
        rp = jnp.transpose(xp, (1, 0, 2)).reshape(NE * BC, CT)
        rb = jnp.transpose(xb, (2, 1, 0, 3)).reshape(R2, CT)      # (nb,s,b)
        R = jnp.concatenate([rc, rp, rb], axis=0)                 # [3072, CT]
        R = jnp.pad(R, ((0, 0), (0, NCHUNK * 128 - CT)))
        R = R.at[:, CT].set(1.0)      # bias col -> chunk 65, partition 80
        T = jnp.transpose(R.reshape(R1 + R2, NCHUNK, 128),
                          (2, 1, 0)).astype(jnp.bfloat16)  # [128, 66, 3072]
        x1 = T[:, :, 0:R1].reshape(128, NCHUNK * R1)
        x2 = T[:, :, R1:R1 + R2].reshape(128, NCHUNK * R2)
        return x1, x2

    cpu = jax.devices("cpu")[0]
    _PREP = (jax.jit(prep), cpu)
    return _PREP


def make_in_maps(Xc, Xp, Xb, W_embed, b_embed, W_ih, W_hh, b_ih, b_hh, W_bil):
    import jax
    import ml_dtypes
    B = Xc.shape[0]
    Xc = np.ascontiguousarray(Xc, np.float32).reshape(B, NE, CT)
    Xp = np.ascontiguousarray(Xp, np.float32).reshape(B, NE, CT)
    Xb = np.ascontiguousarray(Xb, np.float32).reshape(B, NE, NB, CT)

    W_embed = np.ascontiguousarray(W_embed, np.float32)
    W_ch = np.zeros((128, NCHUNK * E), np.float32)
    for j in range(NCHUNK):
        kj = min(128, CT - j * 128)
        W_ch[:kj, j * E:(j + 1) * E] = W_embed[j * 128:j * 128 + kj]
    W_ch[80, (NCHUNK - 1) * E:NCHUNK * E] = b_embed   # folded bias row
    W_ch = W_ch.astype(ml_dtypes.bfloat16)
    WihT = np.ascontiguousarray(W_ih.T, np.float32)          # [100, 300]
    WhhT = np.ascontiguousarray(W_hh.T, np.float32)
    bias4 = np.stack([b_ih[0:E] + b_hh[0:E],
                      b_ih[E:2 * E] + b_hh[E:2 * E],
                      b_ih[2 * E:3 * E],
                      b_hh[2 * E:3 * E]], axis=1).astype(np.float32)
    Wbil_r = np.ascontiguousarray(
        np.transpose(W_bil, (1, 0, 2)).reshape(E, NE * E), np.float32)
    ones = np.ones((E, 1), np.float32)

    shared = dict(Wemb=W_ch, WihT=WihT, WhhT=WhhT, bias4=bias4,
                  Wbil=Wbil_r, ones=ones)
    prep, cpu = _get_prep()
    in_maps = []
    for c in range(N_CORES):
        sl = slice(c * BC, (c + 1) * BC)
        with jax.default_device(cpu):
            x1, x2 = prep(Xc[sl], Xp[sl], Xb[sl])
            x1, x2 = np.asarray(x1), np.asarray(x2)
        in_maps.append(dict(X1=x1, X2=x2, **shared))
    return in_maps


def gather(results):
    outs = []
    for c in range(N_CORES):
        o = results[c]["out"].reshape(-1)
        full = np.empty((BC, NE, NB + 1), np.float32)
        sp = o[0:NE * BC].reshape(NE, BC)                  # [s, b]
        full[:, :, 0] = sp.T
        sb = o[NE * BC:].reshape(NB, NE, BC)               # [nb, s, b]
        full[:, :, 1:] = np.transpose(sb, (2, 1, 0))
        outs.append(full)
    return np.concatenate(outs, axis=0).astype(np.float32)  # [128, 16, 11]


def kernel(Xc, Xp, Xb, W_embed, b_embed, W_ih, W_hh, b_ih, b_hh, W_bil):
    global _NC
    if _NC is None:
        _NC = build()
    in_maps = make_in_maps(Xc, Xp, Xb, W_embed, b_embed, W_ih, W_hh,
                           b_ih, b_hh, W_bil)
    res = run_bass_kernel_spmd(_NC, in_maps, core_ids=list(range(N_CORES)))
    return gather(res.results)


# revision 18
# speedup vs baseline: 1.0013x; 1.0013x over previous
"""CPCNet forward on 8 Trainium2 NeuronCores (Bass/Tile).

Data-parallel over batch: each of the 8 cores processes 16 of the 128
batch elements end-to-end (embed GEMM -> GRU over 16 context windows ->
bilinear scoring), parameters replicated. No collectives needed.

The embed GEMM dominates (memory regime). Two host-side preprocessing
moves cut the device work to a pure DMA stream:
  1. X is cast f32->bf16 on the host, halving HBM traffic
     (103 MB -> 52 MB per core; rel err of bf16 embed ~3e-3).
  2. X is pre-transposed on the host into partition-major chunk layout
     X^T[p, j*rows + r] = X[r, j*128 + p], so the contraction dim (CT)
     lands on partitions. The kernel then needs NO on-chip transposes
     and no PSUM-evacuation copies: the PE just streams 66 chunk
     matmuls per row-group, accumulating E^T directly in PSUM banks.

The embed bias is folded into the GEMM as a virtual 8401st input
column: chunk 65 partition 80 of X^T is 1.0 and the matching W row is
b_embed, so E^T lands in PSUM bias-included.

Per-core layout: rows = [Xc 256 (s*16+b) | Xp 256 | Xb 2560 (nb,s,b)].
Stream 1 (Xc+Xp, 8.45 MB, one DMA) accumulates into PSUM bank A so the
GRU can start ~30 us in; stream 2 (Xb, 43.3 MB, 33 two-chunk DMAs)
accumulates into 5 more PSUM banks while the GRU + bilinear A-matrices
run in the PE's idle gaps. Scoring is a DVE broadcast-multiply against
A plus a ones-matmul column reduction (float32r) at the tail.
"""

import numpy as np

import concourse.bacc as bacc
import concourse.mybir as mybir
import concourse.tile as tile
from concourse.tile import add_dep_helper
from concourse.bass_utils import run_bass_kernel_spmd

N_CORES = 8
BC = 16          # batch per core
NE = 16          # context windows (gru seq len)
NB = 10          # negative samples
CT = 8400        # flattened window (21*400)
E = 100          # embed dim == gru hidden
NCHUNK = 66      # ceil(8448/128); chunk 65 rows 0..79 real, row 80 bias
R1 = 2 * NE * BC            # 512 rows: Xc + Xp
R2 = NB * NE * BC           # 2560 rows: Xb
NPAIR = NCHUNK // 2         # stream-2 DMAs carry 2 chunks each

F32 = mybir.dt.float32
F32R = mybir.dt.float32r
BF16 = mybir.dt.bfloat16


def _emit(nc, tc, ctx):
    X1 = nc.dram_tensor("X1", [128, NCHUNK * R1], BF16, kind="ExternalInput").ap()
    X2 = nc.dram_tensor("X2", [128, NCHUNK * R2], BF16, kind="ExternalInput").ap()
    Wemb = nc.dram_tensor("Wemb", [128, NCHUNK * E], BF16,
                          kind="ExternalInput").ap()
    WihT = nc.dram_tensor("WihT", [E, 300], F32, kind="ExternalInput").ap()
    WhhT = nc.dram_tensor("WhhT", [E, 300], F32, kind="ExternalInput").ap()
    bias4 = nc.dram_tensor("bias4", [E, 4], F32, kind="ExternalInput").ap()
    Wbil = nc.dram_tensor("Wbil", [E, NE * E], F32, kind="ExternalInput").ap()
    ones = nc.dram_tensor("ones", [E, 1], F32R, kind="ExternalInput").ap()
    out_d = nc.dram_tensor("out", [1, R1 // 2 + R2], F32,
                           kind="ExternalOutput").ap()

    P = ctx.enter_context  # pools

    const = P(tc.tile_pool(name="const", bufs=1))
    x1p = P(tc.tile_pool(name="x1p", bufs=4))
    x2p = P(tc.tile_pool(name="x2p", bufs=9))
    psA = P(tc.tile_pool(name="psA", bufs=1, space="PSUM"))
    psT = P(tc.tile_pool(name="psT", bufs=2, space="PSUM"))
    small = P(tc.tile_pool(name="small", bufs=2))

    # ---- persistent SBUF ----
    # W_embed arrives pre-chunked [128, 66*100], pre-cast to bf16, bias
    # folded into chunk 65 row 80: one contiguous 1.7 MB DMA.
    W_sb = const.tile([128, NCHUNK * E], BF16)
    nc.sync.dma_start(W_sb[:], Wemb[:])
    # small params on the other HWDGE ring (scalar) to keep sync FIFO clean
    WihT_sb = const.tile([E, 300], F32)
    nc.scalar.dma_start(WihT_sb[:], WihT[:])
    WhhT_sb = const.tile([E, 300], F32)
    nc.scalar.dma_start(WhhT_sb[:], WhhT[:])
    bias4_sb = const.tile([E, 4], F32)
    nc.scalar.dma_start(bias4_sb[:], bias4[:])
    Wbil_sb = const.tile([E, NE * E], F32)
    nc.scalar.dma_start(Wbil_sb[:], Wbil[:])
    ones_sb = const.tile([E, 1], F32R)
    nc.scalar.dma_start(ones_sb[:], ones[:])

    EcT = const.tile([E, NE * BC], F32)            # Ec^T (bias included)
    gi_sb = const.tile([E, NE * 3 * BC], F32)      # preacts, [s][r|z|n] blocks
    h = const.tile([E, BC], F32)                   # GRU hidden state (h^T)
    A_sb = const.tile([E, NE * BC], F32)           # bilinear A^T, [s][b] cols
    out_sb = const.tile([1, R1 // 2 + R2], F32)

    # Prime the ACT sigmoid/tanh tables during the X1 phase -- each
    # ACT_TABLE_LOAD costs ~1.3 us and would otherwise land mid-GRU on
    # the serial h-chain.
    scr = const.tile([1, 4], F32)
    nc.vector.memset(scr[:], 0.0)
    scr2 = const.tile([1, 4], F32)
    nc.scalar.activation(scr2[:], scr[:], mybir.ActivationFunctionType.Sigmoid)
    scr3 = const.tile([1, 4], F32)
    nc.scalar.activation(scr3[:], scr[:], mybir.ActivationFunctionType.Tanh)

    # ---- PSUM accumulators: 6 full banks, live the whole kernel ----
    bankA = psA.tile([E, R1], F32)                 # Ec^T | Ep^T
    banksB = [psA.tile([E, 512], F32, name=f"bankB{k}") for k in range(5)]

    gi_v = gi_sb.rearrange("e (s g b) -> e s g b", s=NE, g=3)

    def gru_init():
        # gi preacts for all 16 steps in 3 gate matmuls; biases folded
        # (r,z get b_ih+b_hh; n gets b_ih only).  Scattered into the
        # per-step-interleaved gi layout so each step reads one slice.
        nc.vector.memset(h[:], 0.0)
        nc.vector.tensor_copy(EcT[:], bankA[:, 0:NE * BC])
        for g in range(3):
            gp = psT.tile([128, 512], F32, tag="t", name="gp")
            nc.tensor.matmul(gp[0:E, 0:NE * BC], WihT_sb[:, g * E:(g + 1) * E],
                             EcT[:], start=True, stop=True,
                             skip_group_check=True)
            nc.scalar.add(gi_v[:, :, g, :],
                          gp[0:E, 0:NE * BC].rearrange("e (s b) -> e s b", s=NE),
                          bias4_sb[:, g:g + 1])

    def gru_step(s, after_mm=None):
        # DVE only evacuates gh (1 op); elementwise on the idle GpSimd,
        # sigmoid/tanh on ACT.
        c0 = s * 3 * BC
        gh = psT.tile([128, 512], F32, tag="t", name="gh")
        for g in range(3):
            mm = nc.tensor.matmul(gh[0:E, g * BC:(g + 1) * BC],
                                  WhhT_sb[:, g * E:(g + 1) * E], h[:],
                                  start=True, stop=True,
                                  skip_group_check=True)
            if after_mm is not None:
                # pin the step into the PE queue after its pair's matmuls
                # so the scheduler cannot clump consecutive steps (each
                # clump = ~3 us PE idle on the serial h-chain -> HAM K=4/8)
                add_dep_helper(mm.ins, after_mm.ins, sync=False,
                               reason="pin gru step after its pair")
        ghs = small.tile([E, 3 * BC], F32, tag="ghs", name="ghs")
        nc.vector.tensor_copy(ghs[:], gh[0:E, 0:3 * BC])
        rzt = small.tile([E, 2 * BC], F32, tag="rzt", name="rzt")
        nc.gpsimd.tensor_add(rzt[:], ghs[:, 0:2 * BC], gi_sb[:, c0:c0 + 2 * BC])
        rz = small.tile([E, 2 * BC], F32, tag="rz", name="rz")
        nc.scalar.activation(rz[:], rzt[:],
                             mybir.ActivationFunctionType.Sigmoid)
        hn = small.tile([E, BC], F32, tag="hn", name="hn")
        nc.gpsimd.tensor_scalar_add(hn[:], ghs[:, 2 * BC:3 * BC],
                                    bias4_sb[:, 3:4])  # gh_n + b_hn
        t1 = small.tile([E, BC], F32, tag="t1", name="t1")
        nc.gpsimd.tensor_mul(t1[:], rz[:, 0:BC], hn[:])
        t2 = small.tile([E, BC], F32, tag="t2", name="t2")
        nc.gpsimd.tensor_add(t2[:], t1[:], gi_sb[:, c0 + 2 * BC:c0 + 3 * BC])
        n = small.tile([E, BC], F32, tag="n", name="n")
        nc.scalar.activation(n[:], t2[:], mybir.ActivationFunctionType.Tanh)
        d = small.tile([E, BC], F32, tag="d", name="d")
        nc.gpsimd.tensor_sub(d[:], h[:], n[:])
        zd = small.tile([E, BC], F32, tag="zd", name="zd")
        nc.gpsimd.tensor_mul(zd[:], rz[:, BC:2 * BC], d[:])
        nc.gpsimd.tensor_add(h[:], n[:], zd[:])    # h = n + z*(h-n)

    tp = small.tile([E, NE * BC], F32R, tag="tmp0", bufs=1, name="tp")

    def bilinear_A(after_mm=None):
        # A^T[:, s*16+b] = W_bil[s].T @ h^T  for all 16 s into one bank
        Ap = psT.tile([128, 512], F32, tag="t", name="Ap")
        for s in range(NE):
            mm = nc.tensor.matmul(Ap[0:E, s * BC:(s + 1) * BC],
                                  Wbil_sb[:, s * E:(s + 1) * E], h[:],
                                  start=True, stop=True,
                                  skip_group_check=True)
            if after_mm is not None:
                add_dep_helper(mm.ins, after_mm.ins, sync=False,
                               reason="pin bilinear A after its pair")
        nc.vector.tensor_copy(A_sb[:], Ap[0:E, 0:NE * BC])
        # Ep scores can multiply immediately (bankA finished long ago);
        # overlaps the stream-2 tail on the otherwise idle DVE.
        nc.vector.tensor_mul(tp[:], bankA[:, NE * BC:2 * NE * BC], A_sb[:])

    # ---- stream 1: Xc+Xp accumulate into bankA, 6 DMAs of 11 chunks ----
    x1_dmas = []
    for p in range(6):
        xs1 = x1p.tile([128, 11 * R1], BF16, tag="x1", name="xs1")
        x1_dmas.append(nc.sync.dma_start(
            xs1[:], X1[:, p * 11 * R1:(p + 1) * 11 * R1]))
        if p >= 3:
            # cap in-flight x1 DMAs: completion sems lag the data by
            # ~6 us under load, so the issue-dep loop period is
            # (transfer + sem_lag) / cap -- cap 3 keeps it at line rate
            add_dep_helper(x1_dmas[p].ins, x1_dmas[p - 3].ins, sync=True,
                           reason="cap x1 DMA in-flight depth")
        for jj in range(11):
            j = p * 11 + jj
            nc.tensor.matmul(bankA[:, :], W_sb[:, j * E:(j + 1) * E],
                             xs1[:, jj * R1:(jj + 1) * R1],
                             start=(j == 0), stop=(j == NCHUNK - 1),
                             skip_group_check=True)
    gru_init()

    # ---- stream 2: Xb, 33 two-chunk DMAs; one GRU step per pair for
    # the first 16 pairs.  Data drains FIFO within the queue, so each
    # pair's completion sem fires ~6 us (sem receipt latency) after its
    # bytes land: matmuls trail the data stream by ~2 pairs, which both
    # paces the PE (hiding the ~2.9 us serial GRU h-chain) and keeps
    # the tail backlog small.  The in-flight cap must satisfy
    # (transfer + sem_lag) / cap < 3.2 us or the issue-dep loop throttles
    # the stream below line rate (cap 3 measured ~4.1 us/pair). ----
    INFLIGHT = 5
    dma_insts = []
    for pi in range(NPAIR):
        xs = x2p.tile([128, 2 * R2], BF16, tag="x2", name="xs")
        dma_insts.append(nc.sync.dma_start(
            xs[:], X2[:, pi * 2 * R2:(pi + 1) * 2 * R2]))
        if pi >= INFLIGHT:
            add_dep_helper(dma_insts[pi].ins, dma_insts[pi - INFLIGHT].ins,
                           sync=True, reason="cap x2 DMA in-flight depth")
        mms = []
        for u in range(2):
            j = pi * 2 + u
            for k in range(5):
                mms.append(nc.tensor.matmul(
                    banksB[k][:, :], W_sb[:, j * E:(j + 1) * E],
                    xs[:, u * R2 + k * 512:u * R2 + (k + 1) * 512],
                    start=(j == 0), stop=(j == NCHUNK - 1),
                    skip_group_check=True))
        if 1 <= pi <= NE:
            gru_step(pi - 1, after_mm=mms[-1])
        elif pi == NE + 2:
            bilinear_A(after_mm=mms[-1])

    # ---- bilinear scores ----
    # score[r] = A[:, sb(r)] . E^T[:, r]; rows repeat the (s,b) pattern
    # every 256 cols, so the multiplier is A_sb broadcast per 256-block.
    A2 = A_sb.unsqueeze(1).broadcast_to([E, 2, NE * BC])
    SB = NE * BC
    tmps = [(tp, SB)]
    for k in range(5):
        tk = small.tile([E, 512], F32R, tag=f"tmp{k + 1}", bufs=1, name="tk")
        nc.vector.tensor_mul(tk.rearrange("e (i b) -> e i b", i=2),
                             banksB[k][:, :].rearrange("e (i b) -> e i b", i=2),
                             A2)
        tmps.append((tk, 512))
    c0 = 0
    for ti, (tk, w) in enumerate(tmps):
        rp = psT.tile([128, 512], F32, tag="t", name="rp")
        nc.tensor.matmul(rp[0:1, 0:w], ones_sb[:, 0:1], tk[:, 0:w],
                         start=True, stop=True, skip_group_check=True)
        nc.scalar.copy(out_sb[:, c0:c0 + w], rp[0:1, 0:w])
        c0 += w
    nc.sync.dma_start(out_d[:], out_sb[:])


def build():
    import contextlib
    nc = bacc.Bacc("TRN2", target_bir_lowering=False, debug=False,
                   enable_asserts=False, num_devices=N_CORES)
    with tile.TileContext(nc) as tc:
        with contextlib.ExitStack() as ctx:
            _emit(nc, tc, ctx)
    nc.compile()
    return nc


_NC = None
_PREP = None


def _get_prep():
    """jit-compiled per-core X preprocessor on the CPU backend."""
    global _PREP
    if _PREP is not None:
        return _PREP
    import jax
    import jax.numpy as jnp

    def prep(xc, xp, xb):
        # xc/xp [16b,16s,8400], xb [16b,16s,10,8400] (f32)
        rc = jnp.transpose(xc, (1, 0, 2)).reshape(NE * BC, CT)    # (s,b)
        rp = jnp.transpose(xp, (1, 0, 2)).reshape(NE * BC, CT)
        rb = jnp.transpose(xb, (2, 1, 0, 3)).reshape(R2, CT)      # (nb,s,b)
        R = jnp.concatenate([rc, rp, rb], axis=0)                 # [3072, CT]
        R = jnp.pad(R, ((0, 0), (0, NCHUNK * 128 - CT)))
        R = R.at[:, CT].set(1.0)      # bias col -> chunk 65, partition 80
        T = jnp.transpose(R.reshape(R1 + R2, NCHUNK, 128),
                          (2, 1, 0)).astype(jnp.bfloat16)  # [128, 66, 3072]
        x1 = T[:, :, 0:R1].reshape(128, NCHUNK * R1)
        x2 = T[:, :, R1:R1 + R2].reshape(128, NCHUNK * R2)
        return x1, x2

    cpu = jax.devices("cpu")[0]
    _PREP = (jax.jit(prep), cpu)
    return _PREP


def make_in_maps(Xc, Xp, Xb, W_embed, b_embed, W_ih, W_hh, b_ih, b_hh, W_bil):
    import jax
    import ml_dtypes
    B = Xc.shape[0]
    Xc = np.ascontiguousarray(Xc, np.float32).reshape(B, NE, CT)
    Xp = np.ascontiguousarray(Xp, np.float32).reshape(B, NE, CT)
    Xb = np.ascontiguousarray(Xb, np.float32).reshape(B, NE, NB, CT)

    W_embed = np.ascontiguousarray(W_embed, np.float32)
    W_ch = np.zeros((128, NCHUNK * E), np.float32)
    for j in range(NCHUNK):
        kj = min(128, CT - j * 128)
        W_ch[:kj, j * E:(j + 1) * E] = W_embed[j * 128:j * 128 + kj]
    W_ch[80, (NCHUNK - 1) * E:NCHUNK * E] = b_embed   # folded bias row
    W_ch = W_ch.astype(ml_dtypes.bfloat16)
    WihT = np.ascontiguousarray(W_ih.T, np.float32)          # [100, 300]
    WhhT = np.ascontiguousarray(W_hh.T, np.float32)
    bias4 = np.stack([b_ih[0:E] + b_hh[0:E],
                      b_ih[E:2 * E] + b_hh[E:2 * E],
                      b_ih[2 * E:3 * E],
                      b_hh[2 * E:3 * E]], axis=1).astype(np.float32)
    Wbil_r = np.ascontiguousarray(
        np.transpose(W_bil, (1, 0, 2)).reshape(E, NE * E), np.float32)
    ones = np.ones((E, 1), np.float32)

    shared = dict(Wemb=W_ch, WihT=WihT, WhhT=WhhT, bias4=bias4,
                  Wbil=Wbil_r, ones=ones)
    prep, cpu = _get_prep()
    in_maps = []
    for c in range(N_CORES):
        sl = slice(c * BC, (c + 1) * BC)
        with jax.default_device(cpu):
            x1, x2 = prep(Xc[sl], Xp[sl], Xb[sl])
            x1, x2 = np.asarray(x1), np.asarray(x2)
        in_maps.append(dict(X1=x1, X2=x2, **shared))
    return in_maps


def gather(results):
    outs = []
    for c in range(N_CORES):
        o = results[c]["out"].reshape(-1)
        full = np.empty((BC, NE, NB + 1), np.float32)
        sp = o[0:NE * BC].reshape(NE, BC)                  # [s, b]
        full[:, :, 0] = sp.T
        sb = o[NE * BC:].reshape(NB, NE, BC)               # [nb, s, b]
        full[:, :, 1:] = np.transpose(sb, (2, 1, 0))
        outs.append(full)
    return np.concatenate(outs, axis=0).astype(np.float32)  # [128, 16, 11]


def kernel(Xc, Xp, Xb, W_embed, b_embed, W_ih, W_hh, b_ih, b_hh, W_bil):
    global _NC
    if _NC is None:
        _NC = build()
    in_maps = make_in_maps(Xc, Xp, Xb, W_embed, b_embed, W_ih, W_hh,
                           b_ih, b_hh, W_bil)
    res = run_bass_kernel_spmd(_NC, in_maps, core_ids=list(range(N_CORES)))
    return gather(res.results)
